# revision 30
# baseline (speedup 1.0000x reference)
"""NT-Xent contrastive loss on 8 Trainium2 NeuronCores (Bass/Tile), v2.

Strategy (no collectives; slab difference-cover as v1, but fp8 everywhere):
  * Host casts embT and W (x64) to fp8 e4m3. Uniform input scaling is exact:
    the L2 normalize cancels any scalar factor, and the normalize multiplier
    is computed as r = exp(-0.5*ln(normsq) + ln 16) so the normalized
    vectors come out scaled x16 (fp8 sweet spot) regardless of input scale.
    Ln/Exp/Copy share one scalar-engine activation table -> zero table swaps.
  * All big matmuls run fp8 DoubleRow (2 k-subtiles of 128 per pass, 0.5
    cyc/row): head (K=2048 = 8 DR passes), sim blocks (K=256 = 1 DR pass),
    and the colsum ones-matmuls (pairs of exp tiles as the 2 k-subtiles).
  * Off-diagonal exp tiles are written fp8 as exp(10*s - 3) (range safe:
    |s|<=0.45 off-diag), rowsums exact via ACT accum (fp32), colsums from
    the fp8 tiles via DR ones-matmul; host rescales by e^3.
  * Diag-block exp tiles (bf16, contain s=1) are DMA'd to HBM; the host
    extracts the diagonal exp values to subtract self-similarity exactly.
  * pos term: elementwise product of slabs slot0*slot3 + ones-matmul.
  * PSUM budget 16KB/partition: head rotation 2x[128,512] + sim
    double-buffer 2x[128,1024] + smalls rotation 2x[128,512] (normsq rows,
    colsum tiles with results at partition offsets 0/32/64/96).
"""
import math
import numpy as np

SLOTS = [(c, (c + 1) % 8, (c + 2) % 8, (c + 4) % 8) for c in range(8)]
# blocks in local slot coords: (stationary, moving). B0 = diag.
BLOCKS = [(0, 0), (0, 1), (0, 2), (1, 3), (0, 3)]

W_SCALE = 64.0
V_SCALE = 16.0  # normalized vectors scaled x16 into fp8
EXP_BIAS = -3.0  # exp(10*s + EXP_BIAS) keeps off-diag exps in fp8 range

_CACHE = {}


def _build():
    if "nc" in _CACHE:
        return _CACHE["nc"]
    import concourse.bacc as bacc
    import concourse.tile as tile
    import concourse.mybir as mybir

    F32, BF16, F8 = mybir.dt.float32, mybir.dt.bfloat16, mybir.dt.float8e4
    AF = mybir.ActivationFunctionType
    ALU = mybir.AluOpType
    DR = mybir.MatmulPerfMode.DoubleRow

    nc = bacc.Bacc("TRN2", num_devices=8, debug=False)
    a_emb = nc.dram_tensor("embQ", [4, 2, 2, 128, 8, 512], F8,
                       kind="ExternalInput").ap()
    a_W = nc.dram_tensor("Wq", [128, 16, 256], F8, kind="ExternalInput").ap()
    a_b = nc.dram_tensor("b64", [128, 2], F32, kind="ExternalInput").ap()
    a_ones8 = nc.dram_tensor("ones8", [128, 64], F8, kind="ExternalInput").ap()
    a_ones16 = nc.dram_tensor("ones16", [128, 1], BF16, kind="ExternalInput").ap()
    o_rp = nc.dram_tensor("rowpart", [128, 5, 8], F32, kind="ExternalOutput").ap()
    o_cp = nc.dram_tensor("colpart", [1, 8192], F32, kind="ExternalOutput").ap()
    o_dx = nc.dram_tensor("dexp", [8, 128, 1024], BF16, kind="ExternalOutput").ap()
    o_ps = nc.dram_tensor("possim", [1, 1024], F32, kind="ExternalOutput").ap()

    with tile.TileContext(nc) as tc:
        with tc.tile_pool(name="sb", bufs=1) as sb, \
             tc.tile_pool(name="emb", bufs=2) as embp, \
             tc.tile_pool(name="wk", bufs=2) as wk, \
             tc.tile_pool(name="expp", bufs=2) as expp, \
             tc.tile_pool(name="hp", bufs=2, space="PSUM") as hp, \
             tc.tile_pool(name="sp", bufs=2, space="PSUM") as spp, \
             tc.tile_pool(name="smp", bufs=2, space="PSUM") as smp:

            t_W = sb.tile([128, 16, 256], F8, name="t_W")
            nc.sync.dma_start(t_W[:], a_W[:])
            t_b = sb.tile([128, 2], F32, name="t_b")
            ones8 = sb.tile([128, 2, 32], F8, name="ones8")
            ones16 = sb.tile([128, 1], BF16, name="ones16")

            def small_dmas():
                nc.sync.dma_start(t_b[:], a_b[:])
                nc.sync.dma_start(ones8[:],
                                  a_ones8.rearrange("p (c u) -> p c u", u=32))
                nc.sync.dma_start(ones16[:], a_ones16[:])
            LN2_ = 0.6931471805599453
            RB = 0.5 * LN2_ * 126.957 + math.log(V_SCALE)
            t_cb = sb.tile([128, 3], F32, name="t_cb")
            nc.gpsimd.memset(t_cb[:, 0:1], math.log(V_SCALE))
            nc.gpsimd.memset(t_cb[:, 1:2], EXP_BIAS)
            nc.gpsimd.memset(t_cb[:, 2:3], RB)

            # staging
            rp_st = sb.tile([128, 5, 8], F32, name="rp_st")
            cs_st = sb.tile([1, 8192], F32, name="cs_st")
            ps_st = sb.tile([1, 1024], F32, name="ps_st")

            t_on = [sb.tile([128, 2, 1024], F8, name=f"t_on{k}") for k in range(4)]

            LN2 = 0.6931471805599453
            # ln(x) ~= ln2*(bits(x)*2^-23 - 126.957) (mantissa-linear, centered)
            R_SCALE = -0.5 * LN2 / (1 << 23)
            R_BIAS = 0.5 * LN2 * 126.957 + math.log(V_SCALE)

            I32 = mybir.dt.int32

            def stage_chunks(k):
                """Emission closures for stage k; consumed one per mb slot."""
                ctx = {}

                def dma(h, kh):
                    def f():
                        t = embp.tile([128, 8, 512], F8,
                                      name=f"t_e{h}_{kh}", tag="emb", bufs=8)
                        ctx[f"e{h}_{kh}"] = t
                        nc.sync.dma_start(t[:], a_emb[k, h, kh])
                    return f

                def half(h, part):
                    # both dh accumulation chains interleaved -> psum banks
                    # alternate so consecutive matmuls pipeline at full rate
                    def f():
                        if "th" not in ctx:
                            ctx["th"] = wk.tile([128, 2, 1024], BF16,
                                                name="t_h", tag="th")
                        t_e, t_h = ctx[f"e{h}_{part}"], ctx["th"]
                        if part == 0:
                            ctx[f"ph{h}"] = [hp.tile([128, 512], F32,
                                                     name=f"p_h{dh}", tag="hp")
                                             for dh in range(2)]
                        p_h = ctx[f"ph{h}"]
                        for g in range(4 * part, 4 * part + 4):
                            gl = g - 4 * part
                            for dh in range(2):
                                nc.tensor.matmul(
                                    p_h[dh][:],
                                    t_W[:, 2 * g:2 * g + 2,
                                        128 * dh:128 * (dh + 1)],
                                    t_e[:, 2 * gl:2 * gl + 2, :],
                                    start=(g == 0), stop=(g == 7),
                                    perf_mode=DR)
                        if part == 1:
                            for dh in range(2):
                                nc.vector.tensor_scalar_add(
                                    t_h[:, dh, 512 * h:512 * (h + 1)],
                                    p_h[dh][:], t_b[:, dh:dh + 1])
                    return f

                def sqns(nb):
                    def f():
                        t_h = ctx["th"]
                        if nb == 0:
                            ctx["sq"] = wk.tile([128, 2, 1024], BF16,
                                                name="t_sq", tag="sq")
                            ctx["lnb"] = wk.tile([1, 1024], F32, name="t_lnb",
                                                 tag="ln")
                            ctx["r"] = wk.tile([1, 1024], F32, name="t_r",
                                               tag="r")
                        t_sq, t_lnb = ctx["sq"], ctx["lnb"]
                        for dh in range(2):
                            nc.vector.tensor_tensor(
                                t_sq[:, dh, 512 * nb:512 * (nb + 1)],
                                t_h[:, dh, 512 * nb:512 * (nb + 1)],
                                t_h[:, dh, 512 * nb:512 * (nb + 1)], ALU.mult)
                        p_ns = smp.tile([128, 512], F32, name="p_ns", tag="sm")
                        for dh in range(2):
                            nc.tensor.matmul(p_ns[0:1, :], ones16[:],
                                             t_sq[:, dh, 512 * nb:512 * (nb + 1)],
                                             start=(dh == 0), stop=(dh == 1))
                        # float bits of normsq as a number (bit-trick ln)
                        nc.vector.tensor_copy(
                            t_lnb[:, 512 * nb:512 * (nb + 1)],
                            p_ns[0:1, :].bitcast(I32))
                        nc.scalar.activation(
                            ctx["r"][:, 512 * nb:512 * (nb + 1)],
                            t_lnb[:, 512 * nb:512 * (nb + 1)], AF.Exp,
                            scale=R_SCALE, bias=t_cb[0:1, 2:3])
                    return f

                def fin(nb):
                    def f():
                        t_h = ctx["th"]
                        if nb == 0:
                            ctx["bc"] = wk.tile([128, 1024], F32, name="t_bc",
                                                tag="bc")
                        t_bc = ctx["bc"]
                        nc.gpsimd.partition_broadcast(
                            t_bc[:, 512 * nb:512 * (nb + 1)],
                            ctx["r"][:, 512 * nb:512 * (nb + 1)])
                        for dh in range(2):
                            nc.vector.tensor_tensor(
                                t_on[k][:, dh, 512 * nb:512 * (nb + 1)],
                                t_h[:, dh, 512 * nb:512 * (nb + 1)],
                                t_bc[:, 512 * nb:512 * (nb + 1)], ALU.mult)
                    return f

                return ([dma(0, 0), dma(0, 1), dma(1, 0), dma(1, 1)],
                        [half(0, 0), half(0, 1)],
                        [half(1, 0), half(1, 1),
                         sqns(0), sqns(1), fin(0), fin(1)])

            dx_tiles = []

            def dx_dma(i):
                # diag-exp tiles held in SBUF; DMA'd out during B3's
                # otherwise-idle DMA window
                def f():
                    nc.sync.dma_start(o_dx[i], dx_tiles[i][:])
                return f

            def pos_chunks():
                ctx = {}

                def mult(dh, nb):
                    def f():
                        if "pp" not in ctx:
                            ctx["pp"] = wk.tile([128, 2, 1024], BF16,
                                                name="t_pp", tag="sq")
                        nc.vector.tensor_tensor(
                            ctx["pp"][:, dh, 512 * nb:512 * (nb + 1)],
                            t_on[0][:, dh, 512 * nb:512 * (nb + 1)],
                            t_on[3][:, dh, 512 * nb:512 * (nb + 1)], ALU.mult)
                    return f

                def mmcp(nb):
                    def f():
                        p_ps = smp.tile([128, 512], F32, name="p_ps", tag="sm")
                        for dh in range(2):
                            nc.tensor.matmul(
                                p_ps[0:1, :], ones16[:],
                                ctx["pp"][:, dh, 512 * nb:512 * (nb + 1)],
                                start=(dh == 0), stop=(dh == 1))
                        nc.vector.tensor_copy(
                            ps_st[0:1, 512 * nb:512 * (nb + 1)], p_ps[0:1, :])
                    return f

                return [mult(0, 0), mult(1, 0), mmcp(0),
                        mult(0, 1), mult(1, 1), mmcp(1)]

            def block(bslot, a, bm, fuse=()):
                fuse = list(fuse)
                for mb in range(8):
                    p_sim = spp.tile([128, 1024], F32, name="p_sim", tag="sp")
                    for nb in range(2):
                        nc.tensor.matmul(
                            p_sim[:, 512 * nb:512 * (nb + 1)],
                            t_on[a][:, :, 128 * mb:128 * (mb + 1)],
                            t_on[bm][:, :, 512 * nb:512 * (nb + 1)],
                            start=True, stop=True, perf_mode=DR)
                    if bslot == 0:
                        t_e0 = expp.tile([128, 1024], BF16, name="t_e0",
                                         tag="e0", bufs=8)
                        dx_tiles.append(t_e0)
                        nc.scalar.activation(
                            t_e0[:], p_sim[:], AF.Exp,
                            scale=10.0 / (V_SCALE * V_SCALE),
                            bias=t_cb[:, 1:2],
                            accum_out=rp_st[:, 0, mb:mb + 1])
                    else:
                        if mb % 2 == 0:
                            t_ex = expp.tile([128, 2, 1024], F8, name="t_ex",
                                             tag="te")
                        nc.scalar.activation(
                            t_ex[:, mb % 2, :], p_sim[:], AF.Exp,
                            scale=10.0 / (V_SCALE * V_SCALE),
                            bias=t_cb[:, 1:2],
                            accum_out=rp_st[:, bslot, mb:mb + 1])
                        if mb % 2 == 1:
                            pair = mb // 2  # 0..3
                            if pair % 2 == 0:
                                p_cs = [smp.tile([128, 512], F32,
                                                 name=f"p_cs{nb}", tag="sm")
                                        for nb in range(2)]
                            for nb in range(2):
                                nc.tensor.matmul(
                                    p_cs[nb][0:32, :],
                                    ones8[:],
                                    t_ex[:, :, 512 * nb:512 * (nb + 1)],
                                    start=(pair % 2 == 0),
                                    stop=(pair % 2 == 1), perf_mode=DR)
                            if pair % 2 == 1:
                                grp = 2 * (2 * (bslot - 1) + pair // 2)
                                for nb in range(2):
                                    o = 512 * (grp + nb)
                                    nc.vector.tensor_copy(
                                        cs_st[0:1, o:o + 512],
                                        p_cs[nb][0:1, :])
                    if mb < len(fuse):
                        for g in (fuse[mb] if isinstance(fuse[mb], (list, tuple))
                                  else [fuse[mb]]):
                            g()
                for g in fuse[8:]:
                    for gg in (g if isinstance(g, (list, tuple)) else [g]):
                        gg()
                nc.sync.dma_start(o_rp[:, bslot, :], rp_st[:, bslot, :])

            s0_dma, s0_h0, s0_rest = stage_chunks(0)
            s1_dma, s1_h0, s1_rest = stage_chunks(1)
            s2_dma, s2_h0, s2_rest = stage_chunks(2)
            s3_dma, s3_h0, s3_rest = stage_chunks(3)
            for f in s0_dma:
                f()
            small_dmas()
            # warm the PE p-state during the input-DMA wait: throwaway
            # matmuls on t_W keep the array continuously busy so the real
            # head stream starts at full clock
            p_wu = spp.tile([128, 1024], F32, name="p_wu", tag="sp")
            for _ in range(10):
                nc.tensor.matmul(p_wu[:, 0:256],
                                 t_W[:, 0:2, 0:128],
                                 t_W[:, 0:2, 0:256],
                                 start=True, stop=True, perf_mode=DR)
            for f in s1_dma + s0_h0 + s0_rest:
                f()
            b0f = [(s2_dma[0], s1_h0[0]), (s2_dma[1], s1_h0[1]),
                   (s2_dma[2], s1_rest[0]), (s2_dma[3], s1_rest[1]),
                   s1_rest[2], s1_rest[3], s1_rest[4], s1_rest[5]]
            block(0, 0, 0, fuse=b0f)
            b1f = [(s3_dma[0], s2_h0[0]), (s3_dma[1], s2_h0[1]),
                   (s3_dma[2], s2_rest[0]), (s3_dma[3], s2_rest[1]),
                   s2_rest[2], s2_rest[3], s2_rest[4],
                   (s2_rest[5], s3_h0[0], s3_h0[1])]
            block(1, 0, 1, fuse=b1f)
            block(2, 0, 2, fuse=s3_rest)
            block(3, 1, 3, fuse=[dx_dma(i) for i in range(8)])
            block(4, 0, 3, fuse=pos_chunks())

            # final DMAs (layout-matched, no scatter)
            nc.sync.dma_start(o_cp[:], cs_st[:])
            nc.sync.dma_start(o_ps[:], ps_st[:])

    nc.compile()
    _CACHE["nc"] = nc
    return nc


def _host_inputs(embedded_data, W, b):
    import ml_dtypes
    f8 = ml_dtypes.float8_e4m3
    embT = np.asarray(embedded_data, dtype=np.float32).T
    embQ = embT.astype(f8)                     # [2048, 8192]
    Wq = (np.asarray(W, dtype=np.float32) * W_SCALE).astype(f8)
    # device tile layouts, so every DMA is contiguous per partition:
    # Wq_r[p, kc, d] = Wq[128*kc + p, d]
    Wq_r = np.ascontiguousarray(
        Wq.reshape(16, 128, 256).transpose(1, 0, 2))
    b64 = (np.asarray(b, dtype=np.float32) * W_SCALE).reshape(2, 128).T
    b64 = np.ascontiguousarray(b64)
    ones8 = np.ones((128, 64), f8)
    ones16 = np.ones((128, 1), ml_dtypes.bfloat16)
    in_maps = []
    for c in range(8):
        cols = np.concatenate(
            [embQ[:, 1024 * s:1024 * (s + 1)] for s in SLOTS[c]], axis=1)
        # e[k, h, kh, p, c8, r] = cols[1024*kh + 128*c8 + p, 1024*k + 512*h + r]
        e = cols.reshape(2, 8, 128, 4, 2, 512).transpose(3, 4, 0, 2, 1, 5)
        in_maps.append({"embQ": np.ascontiguousarray(e), "Wq": Wq_r,
                        "b64": b64, "ones8": ones8, "ones16": ones16})
    return in_maps


def _combine(results):
    e3 = math.exp(-EXP_BIAS)  # rescale exp(10s-3) -> exp(10s)
    neg = np.zeros(8192, np.float64)
    pos = np.zeros(8192, np.float64)
    idx = np.arange(1024)
    mb_of = idx // 128
    p_of = idx % 128
    for c in range(8):
        S = SLOTS[c]
        rp_st = results[c]["rowpart"].astype(np.float64)  # [128, 5, 8]
        rp = rp_st.transpose(1, 2, 0).reshape(5, 1024)    # [bslot, m*128+p]
        dx = results[c]["dexp"].astype(np.float64)
        cp = results[c]["colpart"].astype(np.float64).reshape(16, 512)
        # diag exp values: sample i (=128*mb+p) at dexp[mb, p, 128*mb+p]
        dg = dx[mb_of, p_of, idx]
        # colsums: cs_st partition group g=2*(2*(B-1)+H)+nb; each entry is
        # the colsum over a half-block (pairs 2H,2H+1) for cols nb*512+[0,512)
        csum = np.zeros((4, 1024), np.float64)
        for B in range(4):
            for H in range(2):
                for nb in range(2):
                    g = 2 * (2 * B + H) + nb
                    csum[B, nb * 512:(nb + 1) * 512] += cp[g]
        sl = [np.s_[1024 * s:1024 * (s + 1)] for s in S]
        neg[sl[0]] += e3 * (rp[0] - dg)            # diag block, self-sim removed
        neg[sl[0]] += e3 * rp[1]; neg[sl[1]] += e3 * csum[0]   # B1 (0,1)
        neg[sl[0]] += e3 * rp[2]; neg[sl[2]] += e3 * csum[1]   # B2 (0,2)
        neg[sl[1]] += e3 * rp[3]; neg[sl[3]] += e3 * csum[2]   # B3 (1,3)
        if c < 4:                                   # B4 (0,3) dedup: cores 0-3
            neg[sl[0]] += e3 * rp[4]; neg[sl[3]] += e3 * csum[3]
            ps = results[c]["possim"].astype(np.float64).ravel()
            ps = ps / (V_SCALE * V_SCALE)
            pos[sl[0]] = ps
            pos[1024 * S[3]:1024 * (S[3] + 1)] = ps
    loss = -np.mean(10.0 * pos - np.log(neg))
    return np.float32(loss)


def run(embedded_data, W, b, trace=False):
    from concourse import bass_utils
    nc = _build()
    in_maps = _host_inputs(embedded_data, W, b)
    res = bass_utils.run_bass_kernel_spmd(nc, in_maps, core_ids=list(range(8)),
                                          trace=trace)
    return _combine(res.results), res


def kernel(embedded_data, W, b):
    loss, _ = run(embedded_data, W, b, trace=False)
    return np.asarray(loss, dtype=np.float32)


# revision 31
# speedup vs baseline: 1.1697x; 1.1697x over previous
"""NT-Xent contrastive loss on 8 Trainium2 NeuronCores (Bass/Tile), v2.

Strategy (no collectives; slab difference-cover as v1, but fp8 everywhere):
  * Host casts embT and W (x64) to fp8 e4m3. Uniform input scaling is exact:
    the L2 normalize cancels any scalar factor, and the normalize multiplier
    is computed as r = exp(-0.5*ln(normsq) + ln 16) so the normalized
    vectors come out scaled x16 (fp8 sweet spot) regardless of input scale.
    Ln/Exp/Copy share one scalar-engine activation table -> zero table swaps.
  * All big matmuls run fp8 DoubleRow (2 k-subtiles of 128 per pass, 0.5
    cyc/row): head (K=2048 = 8 DR passes), sim blocks (K=256 = 1 DR pass),
    and the colsum ones-matmuls (pairs of exp tiles as the 2 k-subtiles).
  * Off-diagonal exp tiles are written fp8 as exp(10*s - 3) (range safe:
    |s|<=0.45 off-diag), rowsums exact via ACT accum (fp32), colsums from
    the fp8 tiles via DR ones-matmul; host rescales by e^3.
  * Diag-block exp tiles (bf16, contain s=1) are DMA'd to HBM; the host
    extracts the diagonal exp values to subtract self-similarity exactly.
  * pos term: elementwise product of slabs slot0*slot3 + ones-matmul.
  * PSUM budget 16KB/partition: head rotation 2x[128,512] + sim
    double-buffer 2x[128,1024] + smalls rotation 2x[128,512] (normsq rows,
    colsum tiles with results at partition offsets 0/32/64/96).
"""
import math
import numpy as np

SLOTS = [(c, (c + 1) % 8, (c + 2) % 8, (c + 4) % 8) for c in range(8)]
# blocks in local slot coords: (stationary, moving). B0 = diag.
BLOCKS = [(0, 0), (0, 1), (0, 2), (1, 3), (0, 3)]

W_SCALE = 64.0
V_SCALE = 16.0  # normalized vectors scaled x16 into fp8
EXP_BIAS = -3.0  # exp(10*s + EXP_BIAS) keeps off-diag exps in fp8 range

_CACHE = {}


def _build():
    if "nc" in _CACHE:
        return _CACHE["nc"]
    import concourse.bacc as bacc
    import concourse.tile as tile
    import concourse.mybir as mybir

    F32, BF16, F8 = mybir.dt.float32, mybir.dt.bfloat16, mybir.dt.float8e4
    AF = mybir.ActivationFunctionType
    ALU = mybir.AluOpType
    DR = mybir.MatmulPerfMode.DoubleRow

    nc = bacc.Bacc("TRN2", num_devices=8, debug=False)
    a_emb = nc.dram_tensor("embQ", [4, 2, 2, 128, 8, 512], F8,
                       kind="ExternalInput").ap()
    a_W = nc.dram_tensor("Wq", [128, 16, 256], F8, kind="ExternalInput").ap()
    a_b = nc.dram_tensor("b64", [128, 2], F32, kind="ExternalInput").ap()
    a_ones8 = nc.dram_tensor("ones8", [128, 64], F8, kind="ExternalInput").ap()
    a_ones16 = nc.dram_tensor("ones16", [128, 1], BF16, kind="ExternalInput").ap()
    o_rp = nc.dram_tensor("rowpart", [128, 5, 8], F32, kind="ExternalOutput").ap()
    o_cp = nc.dram_tensor("colpart", [1, 8192], F32, kind="ExternalOutput").ap()
    o_dx = nc.dram_tensor("dexp", [8, 128, 1024], BF16, kind="ExternalOutput").ap()
    o_ps = nc.dram_tensor("possim", [1, 1024], F32, kind="ExternalOutput").ap()

    with tile.TileContext(nc) as tc:
        with tc.tile_pool(name="sb", bufs=1) as sb, \
             tc.tile_pool(name="emb", bufs=2) as embp, \
             tc.tile_pool(name="wk", bufs=2) as wk, \
             tc.tile_pool(name="expp", bufs=2) as expp, \
             tc.tile_pool(name="hp", bufs=2, space="PSUM") as hp, \
             tc.tile_pool(name="sp", bufs=2, space="PSUM") as spp, \
             tc.tile_pool(name="smp", bufs=2, space="PSUM") as smp:

            t_W = sb.tile([128, 16, 256], F8, name="t_W")
            nc.sync.dma_start(t_W[:], a_W[:])
            t_b = sb.tile([128, 2], F32, name="t_b")
            ones8 = sb.tile([128, 2, 32], F8, name="ones8")
            ones16 = sb.tile([128, 1], BF16, name="ones16")

            def small_dmas():
                nc.sync.dma_start(t_b[:], a_b[:])
                nc.sync.dma_start(ones8[:],
                                  a_ones8.rearrange("p (c u) -> p c u", u=32))
                nc.sync.dma_start(ones16[:], a_ones16[:])
            LN2_ = 0.6931471805599453
            RB = 0.5 * LN2_ * 126.957 + math.log(V_SCALE)
            t_cb = sb.tile([128, 3], F32, name="t_cb")
            nc.gpsimd.memset(t_cb[:, 0:1], math.log(V_SCALE))
            nc.gpsimd.memset(t_cb[:, 1:2], EXP_BIAS)
            nc.gpsimd.memset(t_cb[:, 2:3], RB)

            # staging
            rp_st = sb.tile([128, 5, 8], F32, name="rp_st")
            cs_st = sb.tile([1, 8192], F32, name="cs_st")
            ps_st = sb.tile([1, 1024], F32, name="ps_st")

            t_on = [sb.tile([128, 2, 1024], F8, name=f"t_on{k}") for k in range(4)]

            LN2 = 0.6931471805599453
            # ln(x) ~= ln2*(bits(x)*2^-23 - 126.957) (mantissa-linear, centered)
            R_SCALE = -0.5 * LN2 / (1 << 23)
            R_BIAS = 0.5 * LN2 * 126.957 + math.log(V_SCALE)

            I32 = mybir.dt.int32

            def stage_chunks(k):
                """Emission closures for stage k; consumed one per mb slot."""
                ctx = {}

                def dma(h, kh):
                    def f():
                        t = embp.tile([128, 8, 512], F8,
                                      name=f"t_e{h}_{kh}", tag="emb", bufs=8)
                        ctx[f"e{h}_{kh}"] = t
                        nc.sync.dma_start(t[:], a_emb[k, h, kh])
                    return f

                def half(h, part):
                    # both dh accumulation chains interleaved -> psum banks
                    # alternate so consecutive matmuls pipeline at full rate
                    def f():
                        if "th" not in ctx:
                            ctx["th"] = wk.tile([128, 2, 1024], BF16,
                                                name="t_h", tag="th")
                        t_e, t_h = ctx[f"e{h}_{part}"], ctx["th"]
                        if part == 0:
                            ctx[f"ph{h}"] = [hp.tile([128, 512], F32,
                                                     name=f"p_h{dh}", tag="hp")
                                             for dh in range(2)]
                        p_h = ctx[f"ph{h}"]
                        for g in range(4 * part, 4 * part + 4):
                            gl = g - 4 * part
                            for dh in range(2):
                                nc.tensor.matmul(
                                    p_h[dh][:],
                                    t_W[:, 2 * g:2 * g + 2,
                                        128 * dh:128 * (dh + 1)],
                                    t_e[:, 2 * gl:2 * gl + 2, :],
                                    start=(g == 0), stop=(g == 7),
                                    perf_mode=DR)
                        if part == 1:
                            for dh in range(2):
                                nc.vector.tensor_scalar_add(
                                    t_h[:, dh, 512 * h:512 * (h + 1)],
                                    p_h[dh][:], t_b[:, dh:dh + 1])
                    return f

                def sqns(nb):
                    def f():
                        t_h = ctx["th"]
                        if nb == 0:
                            ctx["sq"] = wk.tile([128, 2, 1024], BF16,
                                                name="t_sq", tag="sq")
                            ctx["lnb"] = wk.tile([1, 1024], F32, name="t_lnb",
                                                 tag="ln")
                            ctx["r"] = wk.tile([1, 1024], F32, name="t_r",
                                               tag="r")
                        t_sq, t_lnb = ctx["sq"], ctx["lnb"]
                        for dh in range(2):
                            nc.vector.tensor_tensor(
                                t_sq[:, dh, 512 * nb:512 * (nb + 1)],
                                t_h[:, dh, 512 * nb:512 * (nb + 1)],
                                t_h[:, dh, 512 * nb:512 * (nb + 1)], ALU.mult)
                        p_ns = smp.tile([128, 512], F32, name="p_ns", tag="sm")
                        for dh in range(2):
                            nc.tensor.matmul(p_ns[0:1, :], ones16[:],
                                             t_sq[:, dh, 512 * nb:512 * (nb + 1)],
                                             start=(dh == 0), stop=(dh == 1))
                        # float bits of normsq as a number (bit-trick ln)
                        nc.vector.tensor_copy(
                            t_lnb[:, 512 * nb:512 * (nb + 1)],
                            p_ns[0:1, :].bitcast(I32))
                        nc.scalar.activation(
                            ctx["r"][:, 512 * nb:512 * (nb + 1)],
                            t_lnb[:, 512 * nb:512 * (nb + 1)], AF.Exp,
                            scale=R_SCALE, bias=t_cb[0:1, 2:3])
                    return f

                def fin(nb):
                    def f():
                        t_h = ctx["th"]
                        if nb == 0:
                            ctx["bc"] = wk.tile([128, 1024], F32, name="t_bc",
                                                tag="bc")
                        t_bc = ctx["bc"]
                        nc.gpsimd.partition_broadcast(
                            t_bc[:, 512 * nb:512 * (nb + 1)],
                            ctx["r"][:, 512 * nb:512 * (nb + 1)])
                        for dh in range(2):
                            nc.vector.tensor_tensor(
                                t_on[k][:, dh, 512 * nb:512 * (nb + 1)],
                                t_h[:, dh, 512 * nb:512 * (nb + 1)],
                                t_bc[:, 512 * nb:512 * (nb + 1)], ALU.mult)
                    return f

                return ([dma(0, 0), dma(0, 1), dma(1, 0), dma(1, 1)],
                        [half(0, 0), half(0, 1)],
                        [half(1, 0), half(1, 1),
                         sqns(0), sqns(1), fin(0), fin(1)])

            dx_tiles = []

            def dx_dma(i):
                # diag-exp tiles held in SBUF; DMA'd out during B3's
                # otherwise-idle DMA window
                def f():
                    nc.sync.dma_start(o_dx[i], dx_tiles[i][:])
                return f

            def pos_chunks():
                ctx = {}

                def mult(dh, nb):
                    def f():
                        if "pp" not in ctx:
                            ctx["pp"] = wk.tile([128, 2, 1024], BF16,
                                                name="t_pp", tag="sq")
                        nc.vector.tensor_tensor(
                            ctx["pp"][:, dh, 512 * nb:512 * (nb + 1)],
                            t_on[0][:, dh, 512 * nb:512 * (nb + 1)],
                            t_on[3][:, dh, 512 * nb:512 * (nb + 1)], ALU.mult)
                    return f

                def mmcp(nb):
                    def f():
                        p_ps = smp.tile([128, 512], F32, name="p_ps", tag="sm")
                        for dh in range(2):
                            nc.tensor.matmul(
                                p_ps[0:1, :], ones16[:],
                                ctx["pp"][:, dh, 512 * nb:512 * (nb + 1)],
                                start=(dh == 0), stop=(dh == 1))
                        nc.vector.tensor_copy(
                            ps_st[0:1, 512 * nb:512 * (nb + 1)], p_ps[0:1, :])
                    return f

                return [mult(0, 0), mult(1, 0), mmcp(0),
                        mult(0, 1), mult(1, 1), mmcp(1)]

            def block(bslot, a, bm, fuse=()):
                fuse = list(fuse)
                for mb in range(8):
                    p_sim = spp.tile([128, 1024], F32, name="p_sim", tag="sp")
                    for nb in range(2):
                        nc.tensor.matmul(
                            p_sim[:, 512 * nb:512 * (nb + 1)],
                            t_on[a][:, :, 128 * mb:128 * (mb + 1)],
                            t_on[bm][:, :, 512 * nb:512 * (nb + 1)],
                            start=True, stop=True, perf_mode=DR)
                    if bslot == 0:
                        t_e0 = expp.tile([128, 1024], BF16, name="t_e0",
                                         tag="e0", bufs=8)
                        dx_tiles.append(t_e0)
                        nc.scalar.activation(
                            t_e0[:], p_sim[:], AF.Exp,
                            scale=10.0 / (V_SCALE * V_SCALE),
                            bias=t_cb[:, 1:2],
                            accum_out=rp_st[:, 0, mb:mb + 1])
                    else:
                        if mb % 2 == 0:
                            t_ex = expp.tile([128, 2, 1024], F8, name="t_ex",
                                             tag="te")
                        nc.scalar.activation(
                            t_ex[:, mb % 2, :], p_sim[:], AF.Exp,
                            scale=10.0 / (V_SCALE * V_SCALE),
                            bias=t_cb[:, 1:2],
                            accum_out=rp_st[:, bslot, mb:mb + 1])
                        if mb % 2 == 1:
                            pair = mb // 2  # 0..3
                            if pair % 2 == 0:
                                p_cs = [smp.tile([128, 512], F32,
                                                 name=f"p_cs{nb}", tag="sm")
                                        for nb in range(2)]
                            for nb in range(2):
                                nc.tensor.matmul(
                                    p_cs[nb][0:32, :],
                                    ones8[:],
                                    t_ex[:, :, 512 * nb:512 * (nb + 1)],
                                    start=(pair % 2 == 0),
                                    stop=(pair % 2 == 1), perf_mode=DR)
                            if pair % 2 == 1:
                                grp = 2 * (2 * (bslot - 1) + pair // 2)
                                for nb in range(2):
                                    o = 512 * (grp + nb)
                                    nc.vector.tensor_copy(
                                        cs_st[0:1, o:o + 512],
                                        p_cs[nb][0:1, :])
                    if mb < len(fuse):
                        for g in (fuse[mb] if isinstance(fuse[mb], (list, tuple))
                                  else [fuse[mb]]):
                            g()
                for g in fuse[8:]:
                    for gg in (g if isinstance(g, (list, tuple)) else [g]):
                        gg()
                nc.sync.dma_start(o_rp[:, bslot, :], rp_st[:, bslot, :])

            s0_dma, s0_h0, s0_rest = stage_chunks(0)
            s1_dma, s1_h0, s1_rest = stage_chunks(1)
            s2_dma, s2_h0, s2_rest = stage_chunks(2)
            s3_dma, s3_h0, s3_rest = stage_chunks(3)
            for f in s0_dma:
                f()
            small_dmas()
            for f in s1_dma + s0_h0 + s0_rest:
                f()
            b0f = [(s2_dma[0], s1_h0[0]), (s2_dma[1], s1_h0[1]),
                   (s2_dma[2], s1_rest[0]), (s2_dma[3], s1_rest[1]),
                   s1_rest[2], s1_rest[3], s1_rest[4], s1_rest[5]]
            block(0, 0, 0, fuse=b0f)
            b1f = [(s3_dma[0], s2_h0[0]), (s3_dma[1], s2_h0[1]),
                   (s3_dma[2], s2_rest[0]), (s3_dma[3], s2_rest[1]),
                   s2_rest[2], s2_rest[3], s2_rest[4],
                   (s2_rest[5], s3_h0[0], s3_h0[1])]
            block(1, 0, 1, fuse=b1f)
            block(2, 0, 2, fuse=s3_rest)
            block(3, 1, 3, fuse=[dx_dma(i) for i in range(8)])
            block(4, 0, 3, fuse=pos_chunks())

            # final DMAs (layout-matched, no scatter)
            nc.sync.dma_start(o_cp[:], cs_st[:])
            nc.sync.dma_start(o_ps[:], ps_st[:])

    nc.compile()
    _CACHE["nc"] = nc
    return nc


def _host_inputs(embedded_data, W, b):
    import ml_dtypes
    f8 = ml_dtypes.float8_e4m3
    embT = np.asarray(embedded_data, dtype=np.float32).T
    embQ = embT.astype(f8)                     # [2048, 8192]
    Wq = (np.asarray(W, dtype=np.float32) * W_SCALE).astype(f8)
    # device tile layouts, so every DMA is contiguous per partition:
    # Wq_r[p, kc, d] = Wq[128*kc + p, d]
    Wq_r = np.ascontiguousarray(
        Wq.reshape(16, 128, 256).transpose(1, 0, 2))
    b64 = (np.asarray(b, dtype=np.float32) * W_SCALE).reshape(2, 128).T
    b64 = np.ascontiguousarray(b64)
    ones8 = np.ones((128, 64), f8)
    ones16 = np.ones((128, 1), ml_dtypes.bfloat16)
    in_maps = []
    for c in range(8):
        cols = np.concatenate(
            [embQ[:, 1024 * s:1024 * (s + 1)] for s in SLOTS[c]], axis=1)
        # e[k, h, kh, p, c8, r] = cols[1024*kh + 128*c8 + p, 1024*k + 512*h + r]
        e = cols.reshape(2, 8, 128, 4, 2, 512).transpose(3, 4, 0, 2, 1, 5)
        in_maps.append({"embQ": np.ascontiguousarray(e), "Wq": Wq_r,
                        "b64": b64, "ones8": ones8, "ones16": ones16})
    return in_maps


def _combine(results):
    e3 = math.exp(-EXP_BIAS)  # rescale exp(10s-3) -> exp(10s)
    neg = np.zeros(8192, np.float64)
    pos = np.zeros(8192, np.float64)
    idx = np.arange(1024)
    mb_of = idx // 128
    p_of = idx % 128
    for c in range(8):
        S = SLOTS[c]
        rp_st = results[c]["rowpart"].astype(np.float64)  # [128, 5, 8]
        rp = rp_st.transpose(1, 2, 0).reshape(5, 1024)    # [bslot, m*128+p]
        dx = results[c]["dexp"].astype(np.float64)
        cp = results[c]["colpart"].astype(np.float64).reshape(16, 512)
        # diag exp values: sample i (=128*mb+p) at dexp[mb, p, 128*mb+p]
        dg = dx[mb_of, p_of, idx]
        # colsums: cs_st partition group g=2*(2*(B-1)+H)+nb; each entry is
        # the colsum over a half-block (pairs 2H,2H+1) for cols nb*512+[0,512)
        csum = np.zeros((4, 1024), np.float64)
        for B in range(4):
            for H in range(2):
                for nb in range(2):
                    g = 2 * (2 * B + H) + nb
                    csum[B, nb * 512:(nb + 1) * 512] += cp[g]
        sl = [np.s_[1024 * s:1024 * (s + 1)] for s in S]
        neg[sl[0]] += e3 * (rp[0] - dg)            # diag block, self-sim removed
        neg[sl[0]] += e3 * rp[1]; neg[sl[1]] += e3 * csum[0]   # B1 (0,1)
        neg[sl[0]] += e3 * rp[2]; neg[sl[2]] += e3 * csum[1]   # B2 (0,2)
        neg[sl[1]] += e3 * rp[3]; neg[sl[3]] += e3 * csum[2]   # B3 (1,3)
        if c < 4:                                   # B4 (0,3) dedup: cores 0-3
            neg[sl[0]] += e3 * rp[4]; neg[sl[3]] += e3 * csum[3]
            ps = results[c]["possim"].astype(np.float64).ravel()
            ps = ps / (V_SCALE * V_SCALE)
            pos[sl[0]] = ps
            pos[1024 * S[3]:1024 * (S[3] + 1)] = ps
    loss = -np.mean(10.0 * pos - np.log(neg))
    return np.float32(loss)


def run(embedded_data, W, b, trace=False):
    from concourse import bass_utils
    nc = _build()
    in_maps = _host_inputs(embedded_data, W, b)
    res = bass_utils.run_bass_kernel_spmd(nc, in_maps, core_ids=list(range(8)),
                                          trace=trace)
    return _combine(res.results), res


def kernel(embedded_data, W, b):
    loss, _ = run(embedded_data, W, b, trace=False)
    return np.asarray(loss, dtype=np.float32)


# revision 32
# speedup vs baseline: 1.1888x; 1.0164x over previous
"""NT-Xent contrastive loss on 8 Trainium2 NeuronCores (Bass/Tile), v2.

Strategy (no collectives; slab difference-cover as v1, but fp8 everywhere):
  * Host casts embT and W (x64) to fp8 e4m3. Uniform input scaling is exact:
    the L2 normalize cancels any scalar factor, and the normalize multiplier
    is computed as r = exp(-0.5*ln(normsq) + ln 16) so the normalized
    vectors come out scaled x16 (fp8 sweet spot) regardless of input scale.
    Ln/Exp/Copy share one scalar-engine activation table -> zero table swaps.
  * All big matmuls run fp8 DoubleRow (2 k-subtiles of 128 per pass, 0.5
    cyc/row): head (K=2048 = 8 DR passes), sim blocks (K=256 = 1 DR pass),
    and the colsum ones-matmuls (pairs of exp tiles as the 2 k-subtiles).
  * Off-diagonal exp tiles are written fp8 as exp(10*s - 3) (range safe:
    |s|<=0.45 off-diag), rowsums exact via ACT accum (fp32), colsums from
    the fp8 tiles via DR ones-matmul; host rescales by e^3.
  * Diag-block exp tiles (bf16, contain s=1) are DMA'd to HBM; the host
    extracts the diagonal exp values to subtract self-similarity exactly.
  * pos term: elementwise product of slabs slot0*slot3 + ones-matmul.
  * PSUM budget 16KB/partition: head rotation 2x[128,512] + sim
    double-buffer 2x[128,1024] + smalls rotation 2x[128,512] (normsq rows,
    colsum tiles with results at partition offsets 0/32/64/96).
"""
import math
import numpy as np

SLOTS = [(c, (c + 1) % 8, (c + 2) % 8, (c + 4) % 8) for c in range(8)]
# blocks in local slot coords: (stationary, moving). B0 = diag.
BLOCKS = [(0, 0), (0, 1), (0, 2), (1, 3), (0, 3)]

W_SCALE = 64.0
V_SCALE = 16.0  # normalized vectors scaled x16 into fp8
EXP_BIAS = -3.0  # exp(10*s + EXP_BIAS) keeps off-diag exps in fp8 range

_CACHE = {}


def _build():
    if "nc" in _CACHE:
        return _CACHE["nc"]
    import concourse.bacc as bacc
    import concourse.tile as tile
    import concourse.mybir as mybir

    F32, BF16, F8 = mybir.dt.float32, mybir.dt.bfloat16, mybir.dt.float8e4
    AF = mybir.ActivationFunctionType
    ALU = mybir.AluOpType
    DR = mybir.MatmulPerfMode.DoubleRow

    nc = bacc.Bacc("TRN2", num_devices=8, debug=False)
    a_emb = nc.dram_tensor("embQ", [4, 2, 2, 128, 8, 512], F8,
                       kind="ExternalInput").ap()
    a_W = nc.dram_tensor("Wq", [128, 16, 256], F8, kind="ExternalInput").ap()
    a_b = nc.dram_tensor("b64", [128, 2], F32, kind="ExternalInput").ap()
    a_ones8 = nc.dram_tensor("ones8", [128, 64], F8, kind="ExternalInput").ap()
    a_ones16 = nc.dram_tensor("ones16", [128, 1], BF16, kind="ExternalInput").ap()
    o_rp = nc.dram_tensor("rowpart", [128, 5, 8], F32, kind="ExternalOutput").ap()
    o_cp = nc.dram_tensor("colpart", [1, 8192], F32, kind="ExternalOutput").ap()
    o_dx = nc.dram_tensor("dexp", [8, 128, 1024], BF16, kind="ExternalOutput").ap()
    o_ps = nc.dram_tensor("possim", [1, 1024], F32, kind="ExternalOutput").ap()

    with tile.TileContext(nc) as tc:
        with tc.tile_pool(name="sb", bufs=1) as sb, \
             tc.tile_pool(name="emb", bufs=2) as embp, \
             tc.tile_pool(name="wk", bufs=2) as wk, \
             tc.tile_pool(name="expp", bufs=2) as expp, \
             tc.tile_pool(name="hp", bufs=2, space="PSUM") as hp, \
             tc.tile_pool(name="sp", bufs=2, space="PSUM") as spp, \
             tc.tile_pool(name="smp", bufs=2, space="PSUM") as smp:

            t_W = sb.tile([128, 16, 256], F8, name="t_W")
            nc.sync.dma_start(t_W[:], a_W[:])
            t_b = sb.tile([128, 2], F32, name="t_b")
            ones8 = sb.tile([128, 2, 32], F8, name="ones8")
            ones16 = sb.tile([128, 1], BF16, name="ones16")

            def small_dmas():
                nc.sync.dma_start(t_b[:], a_b[:])
                nc.sync.dma_start(ones8[:],
                                  a_ones8.rearrange("p (c u) -> p c u", u=32))
                nc.sync.dma_start(ones16[:], a_ones16[:])
            LN2_ = 0.6931471805599453
            RB = 0.5 * LN2_ * 126.957 + math.log(V_SCALE)
            t_cb = sb.tile([128, 3], F32, name="t_cb")
            nc.gpsimd.memset(t_cb[:, 0:1], math.log(V_SCALE))
            nc.gpsimd.memset(t_cb[:, 1:2], EXP_BIAS)
            nc.gpsimd.memset(t_cb[:, 2:3], RB)

            # staging
            rp_st = sb.tile([128, 5, 8], F32, name="rp_st")
            cs_st = sb.tile([1, 8192], F32, name="cs_st")
            ps_st = sb.tile([1, 1024], F32, name="ps_st")

            t_on = [sb.tile([128, 2, 1024], F8, name=f"t_on{k}") for k in range(4)]

            LN2 = 0.6931471805599453
            # ln(x) ~= ln2*(bits(x)*2^-23 - 126.957) (mantissa-linear, centered)
            R_SCALE = -0.5 * LN2 / (1 << 23)
            R_BIAS = 0.5 * LN2 * 126.957 + math.log(V_SCALE)

            I32 = mybir.dt.int32

            def stage_chunks(k):
                """Emission closures for stage k; consumed one per mb slot."""
                ctx = {}

                def dma(h, kh):
                    def f():
                        t = embp.tile([128, 8, 512], F8,
                                      name=f"t_e{h}_{kh}", tag="emb", bufs=8)
                        ctx[f"e{h}_{kh}"] = t
                        nc.sync.dma_start(t[:], a_emb[k, h, kh])
                    return f

                def half(h, part):
                    # both dh accumulation chains interleaved -> psum banks
                    # alternate so consecutive matmuls pipeline at full rate
                    def f():
                        if "th" not in ctx:
                            ctx["th"] = wk.tile([128, 2, 1024], BF16,
                                                name="t_h", tag="th")
                        t_e, t_h = ctx[f"e{h}_{part}"], ctx["th"]
                        if part == 0:
                            ctx[f"ph{h}"] = [hp.tile([128, 512], F32,
                                                     name=f"p_h{dh}", tag="hp")
                                             for dh in range(2)]
                        p_h = ctx[f"ph{h}"]
                        for g in range(4 * part, 4 * part + 4):
                            gl = g - 4 * part
                            for dh in range(2):
                                nc.tensor.matmul(
                                    p_h[dh][:],
                                    t_W[:, 2 * g:2 * g + 2,
                                        128 * dh:128 * (dh + 1)],
                                    t_e[:, 2 * gl:2 * gl + 2, :],
                                    start=(g == 0), stop=(g == 7),
                                    perf_mode=DR)
                        if part == 1:
                            for dh in range(2):
                                nc.vector.tensor_scalar_add(
                                    t_h[:, dh, 512 * h:512 * (h + 1)],
                                    p_h[dh][:], t_b[:, dh:dh + 1])
                    return f

                def sqns(nb):
                    def f():
                        t_h = ctx["th"]
                        if nb == 0:
                            ctx["sq"] = wk.tile([128, 2, 1024], BF16,
                                                name="t_sq", tag="sq")
                            ctx["lnb"] = wk.tile([1, 1024], F32, name="t_lnb",
                                                 tag="ln")
                            ctx["r"] = wk.tile([1, 1024], F32, name="t_r",
                                               tag="r")
                        t_sq, t_lnb = ctx["sq"], ctx["lnb"]
                        for dh in range(2):
                            nc.vector.tensor_tensor(
                                t_sq[:, dh, 512 * nb:512 * (nb + 1)],
                                t_h[:, dh, 512 * nb:512 * (nb + 1)],
                                t_h[:, dh, 512 * nb:512 * (nb + 1)], ALU.mult)
                        p_ns = smp.tile([128, 512], F32, name="p_ns", tag="sm")
                        for dh in range(2):
                            nc.tensor.matmul(p_ns[0:1, :], ones16[:],
                                             t_sq[:, dh, 512 * nb:512 * (nb + 1)],
                                             start=(dh == 0), stop=(dh == 1))
                        # float bits of normsq as a number (bit-trick ln)
                        nc.vector.tensor_copy(
                            t_lnb[:, 512 * nb:512 * (nb + 1)],
                            p_ns[0:1, :].bitcast(I32))
                        nc.scalar.activation(
                            ctx["r"][:, 512 * nb:512 * (nb + 1)],
                            t_lnb[:, 512 * nb:512 * (nb + 1)], AF.Exp,
                            scale=R_SCALE, bias=t_cb[0:1, 2:3])
                    return f

                def fin(nb):
                    def f():
                        t_h = ctx["th"]
                        if nb == 0:
                            ctx["bc"] = wk.tile([128, 1024], F32, name="t_bc",
                                                tag="bc")
                        t_bc = ctx["bc"]
                        nc.gpsimd.partition_broadcast(
                            t_bc[:, 512 * nb:512 * (nb + 1)],
                            ctx["r"][:, 512 * nb:512 * (nb + 1)])
                        for dh in range(2):
                            nc.vector.tensor_tensor(
                                t_on[k][:, dh, 512 * nb:512 * (nb + 1)],
                                t_h[:, dh, 512 * nb:512 * (nb + 1)],
                                t_bc[:, 512 * nb:512 * (nb + 1)], ALU.mult)
                    return f

                return ([dma(0, 0), dma(0, 1), dma(1, 0), dma(1, 1)],
                        [half(0, 0), half(0, 1)],
                        [half(1, 0), half(1, 1),
                         sqns(0), sqns(1), fin(0), fin(1)])

            dx_tiles = []

            def dx_dma(i):
                # diag-exp tiles held in SBUF; DMA'd out during B3's
                # otherwise-idle DMA window
                def f():
                    nc.sync.dma_start(o_dx[i], dx_tiles[i][:])
                return f

            def pos_chunks():
                ctx = {}

                def mult(dh, nb):
                    def f():
                        if "pp" not in ctx:
                            ctx["pp"] = wk.tile([128, 2, 1024], BF16,
                                                name="t_pp", tag="sq")
                        nc.vector.tensor_tensor(
                            ctx["pp"][:, dh, 512 * nb:512 * (nb + 1)],
                            t_on[0][:, dh, 512 * nb:512 * (nb + 1)],
                            t_on[3][:, dh, 512 * nb:512 * (nb + 1)], ALU.mult)
                    return f

                def mmcp(nb):
                    def f():
                        p_ps = smp.tile([128, 512], F32, name="p_ps", tag="sm")
                        for dh in range(2):
                            nc.tensor.matmul(
                                p_ps[0:1, :], ones16[:],
                                ctx["pp"][:, dh, 512 * nb:512 * (nb + 1)],
                                start=(dh == 0), stop=(dh == 1))
                        nc.vector.tensor_copy(
                            ps_st[0:1, 512 * nb:512 * (nb + 1)], p_ps[0:1, :])
                    return f

                return [mult(0, 0), mult(1, 0), mmcp(0),
                        mult(0, 1), mult(1, 1), mmcp(1)]

            def block(bslot, a, bm, fuse=()):
                fuse = list(fuse)
                for mb in range(8):
                    p_sim = spp.tile([128, 1024], F32, name="p_sim", tag="sp")
                    for nb in range(2):
                        nc.tensor.matmul(
                            p_sim[:, 512 * nb:512 * (nb + 1)],
                            t_on[a][:, :, 128 * mb:128 * (mb + 1)],
                            t_on[bm][:, :, 512 * nb:512 * (nb + 1)],
                            start=True, stop=True, perf_mode=DR)
                    if bslot == 0:
                        t_e0 = expp.tile([128, 1024], BF16, name="t_e0",
                                         tag="e0", bufs=8)
                        dx_tiles.append(t_e0)
                        nc.scalar.activation(
                            t_e0[:], p_sim[:], AF.Exp,
                            scale=10.0 / (V_SCALE * V_SCALE),
                            bias=t_cb[:, 1:2],
                            accum_out=rp_st[:, 0, mb:mb + 1])
                    else:
                        if mb % 2 == 0:
                            t_ex = expp.tile([128, 2, 1024], F8, name="t_ex",
                                             tag="te")
                        nc.scalar.activation(
                            t_ex[:, mb % 2, :], p_sim[:], AF.Exp,
                            scale=10.0 / (V_SCALE * V_SCALE),
                            bias=t_cb[:, 1:2],
                            accum_out=rp_st[:, bslot, mb:mb + 1])
                        if mb % 2 == 1:
                            pair = mb // 2  # 0..3
                            if pair % 2 == 0:
                                p_cs = [smp.tile([128, 512], F32,
                                                 name=f"p_cs{nb}", tag="sm")
                                        for nb in range(2)]
                            for nb in range(2):
                                nc.tensor.matmul(
                                    p_cs[nb][0:32, :],
                                    ones8[:],
                                    t_ex[:, :, 512 * nb:512 * (nb + 1)],
                                    start=(pair % 2 == 0),
                                    stop=(pair % 2 == 1), perf_mode=DR)
                            if pair % 2 == 1:
                                grp = 2 * (2 * (bslot - 1) + pair // 2)
                                for nb in range(2):
                                    o = 512 * (grp + nb)
                                    nc.vector.tensor_copy(
                                        cs_st[0:1, o:o + 512],
                                        p_cs[nb][0:1, :])
                    if mb < len(fuse):
                        for g in (fuse[mb] if isinstance(fuse[mb], (list, tuple))
                                  else [fuse[mb]]):
                            g()
                for g in fuse[8:]:
                    for gg in (g if isinstance(g, (list, tuple)) else [g]):
                        gg()
                nc.sync.dma_start(o_rp[:, bslot, :], rp_st[:, bslot, :])
                if bslot > 0:
                    o = 2048 * (bslot - 1)
                    nc.sync.dma_start(o_cp[0:1, o:o + 2048],
                                      cs_st[0:1, o:o + 2048])

            s0_dma, s0_h0, s0_rest = stage_chunks(0)
            s1_dma, s1_h0, s1_rest = stage_chunks(1)
            s2_dma, s2_h0, s2_rest = stage_chunks(2)
            s3_dma, s3_h0, s3_rest = stage_chunks(3)
            for f in s0_dma:
                f()
            small_dmas()
            for f in s1_dma + s0_h0 + s0_rest:
                f()
            b0f = [(s2_dma[0], s1_h0[0]), (s2_dma[1], s1_h0[1]),
                   (s2_dma[2], s1_rest[0]), (s2_dma[3], s1_rest[1]),
                   s1_rest[2], s1_rest[3], s1_rest[4], s1_rest[5]]
            block(0, 0, 0, fuse=b0f)
            b1f = [(s3_dma[0], s2_h0[0]), (s3_dma[1], s2_h0[1]),
                   (s3_dma[2], s2_rest[0]), (s3_dma[3], s2_rest[1]),
                   s2_rest[2], (s2_rest[3], s3_h0[0]),
                   (s2_rest[4], s3_h0[1]), s2_rest[5]]
            block(1, 0, 1, fuse=b1f)
            block(2, 0, 2, fuse=s3_rest)
            block(3, 1, 3, fuse=[dx_dma(i) for i in range(8)])
            block(4, 0, 3, fuse=pos_chunks())

            # final DMA (colpart/rowpart streamed per block)
            nc.sync.dma_start(o_ps[:], ps_st[:])

    nc.compile()
    _CACHE["nc"] = nc
    return nc


def _host_inputs(embedded_data, W, b):
    import ml_dtypes
    f8 = ml_dtypes.float8_e4m3
    embT = np.asarray(embedded_data, dtype=np.float32).T
    embQ = embT.astype(f8)                     # [2048, 8192]
    Wq = (np.asarray(W, dtype=np.float32) * W_SCALE).astype(f8)
    # device tile layouts, so every DMA is contiguous per partition:
    # Wq_r[p, kc, d] = Wq[128*kc + p, d]
    Wq_r = np.ascontiguousarray(
        Wq.reshape(16, 128, 256).transpose(1, 0, 2))
    b64 = (np.asarray(b, dtype=np.float32) * W_SCALE).reshape(2, 128).T
    b64 = np.ascontiguousarray(b64)
    ones8 = np.ones((128, 64), f8)
    ones16 = np.ones((128, 1), ml_dtypes.bfloat16)
    in_maps = []
    for c in range(8):
        cols = np.concatenate(
            [embQ[:, 1024 * s:1024 * (s + 1)] for s in SLOTS[c]], axis=1)
        # e[k, h, kh, p, c8, r] = cols[1024*kh + 128*c8 + p, 1024*k + 512*h + r]
        e = cols.reshape(2, 8, 128, 4, 2, 512).transpose(3, 4, 0, 2, 1, 5)
        in_maps.append({"embQ": np.ascontiguousarray(e), "Wq": Wq_r,
                        "b64": b64, "ones8": ones8, "ones16": ones16})
    return in_maps


def _combine(results):
    e3 = math.exp(-EXP_BIAS)  # rescale exp(10s-3) -> exp(10s)
    neg = np.zeros(8192, np.float64)
    pos = np.zeros(8192, np.float64)
    idx = np.arange(1024)
    mb_of = idx // 128
    p_of = idx % 128
    for c in range(8):
        S = SLOTS[c]
        rp_st = results[c]["rowpart"].astype(np.float64)  # [128, 5, 8]
        rp = rp_st.transpose(1, 2, 0).reshape(5, 1024)    # [bslot, m*128+p]
        dx = results[c]["dexp"].astype(np.float64)
        cp = results[c]["colpart"].astype(np.float64).reshape(16, 512)
        # diag exp values: sample i (=128*mb+p) at dexp[mb, p, 128*mb+p]
        dg = dx[mb_of, p_of, idx]
        # colsums: cs_st partition group g=2*(2*(B-1)+H)+nb; each entry is
        # the colsum over a half-block (pairs 2H,2H+1) for cols nb*512+[0,512)
        csum = np.zeros((4, 1024), np.float64)
        for B in range(4):
            for H in range(2):
                for nb in range(2):
                    g = 2 * (2 * B + H) + nb
                    csum[B, nb * 512:(nb + 1) * 512] += cp[g]
        sl = [np.s_[1024 * s:1024 * (s + 1)] for s in S]
        neg[sl[0]] += e3 * (rp[0] - dg)            # diag block, self-sim removed
        neg[sl[0]] += e3 * rp[1]; neg[sl[1]] += e3 * csum[0]   # B1 (0,1)
        neg[sl[0]] += e3 * rp[2]; neg[sl[2]] += e3 * csum[1]   # B2 (0,2)
        neg[sl[1]] += e3 * rp[3]; neg[sl[3]] += e3 * csum[2]   # B3 (1,3)
        if c < 4:                                   # B4 (0,3) dedup: cores 0-3
            neg[sl[0]] += e3 * rp[4]; neg[sl[3]] += e3 * csum[3]
            ps = results[c]["possim"].astype(np.float64).ravel()
            ps = ps / (V_SCALE * V_SCALE)
            pos[sl[0]] = ps
            pos[1024 * S[3]:1024 * (S[3] + 1)] = ps
    loss = -np.mean(10.0 * pos - np.log(neg))
    return np.float32(loss)


def run(embedded_data, W, b, trace=False):
    from concourse import bass_utils
    nc = _build()
    in_maps = _host_inputs(embedded_data, W, b)
    res = bass_utils.run_bass_kernel_spmd(nc, in_maps, core_ids=list(range(8)),
                                          trace=trace)
    return _combine(res.results), res


def kernel(embedded_data, W, b):
    loss, _ = run(embedded_data, W, b, trace=False)
    return np.asarray(loss, dtype=np.float32)
